# revision 1
# baseline (speedup 1.0000x reference)
"""GroupedQueryAttention on 8 Trainium2 NeuronCores.

Problem (hardcoded): B=2, T=2048, DIM=4096, 32 q heads, 8 kv heads, hd=128.
  q = x @ Wq.T ; k,v = split(x @ Wkv.T) ; causal softmax(q k^T/sqrt(hd)) v ; out = o @ Wo.T

Sharding: hybrid data x tensor parallel over 8 cores.
  core c -> batch b = c//4, kv-head group j = c%4 (kv heads {2j,2j+1}, q heads {8j..8j+7}).
Per core:
  phase 1: QT[e,t], KT[dk,t], VT[dv,t] projections (weights pre-transposed on host,
           x pre-transposed on host; all matmul inputs bf16, PSUM f32).
  phase 2: flash-style causal attention per q head in scores-TRANSPOSED layout
           sT[k,q] = KT_tile.T @ QT  (so the AV matmul takes exp(sT) directly as the
           moving operand and V[t,dv] as stationary - no P transposes).
           Softmax denominators via a ones[128,1] matmul (partition-dim reduction).
  phase 3: AllGather oT over the 4-core batch group (bf16), then each core computes
           a 1024-row slice of outT = Wo @ oT, written as f32.
Host: sums nothing - output slices are disjoint; just transpose/concat.
"""

import sys

sys.path.insert(0, "/opt/trn_rl_repo")

import math

import numpy as np

import concourse.bass as bass
import concourse.bacc as bacc
import concourse.tile as tile
from concourse import mybir
from concourse.bass_utils import run_bass_kernel_spmd

B, T, DIM = 2, 2048, 4096
N_HEADS, N_KV, HD = 32, 8, 128
R = N_HEADS // N_KV  # 4
NCORES = 8
GROUPS = [[0, 1, 2, 3], [4, 5, 6, 7]]

HPC = 8  # q heads per core
KVPC = 2  # kv heads per core
EQ = HPC * HD  # 1024 q-proj out features per core
EKV = KVPC * HD  # 256 k (and v) out features per core
NT = T // 512  # 4 t-groups of 512
NC = DIM // 128  # 32 contraction tiles
NKB = T // 128  # 16 k-tiles per head

BF = mybir.dt.bfloat16
F32 = mybir.dt.float32
INV_SQRT_HD = 1.0 / math.sqrt(HD)


def build():
    nc = bacc.Bacc("TRN2", num_devices=NCORES)

    # ---- external I/O (per-core data differs, program is SPMD-identical) ----
    xT = nc.dram_tensor("xT", [DIM, T], BF, kind="ExternalInput")  # x[b].T
    wallT = nc.dram_tensor("wallT", [DIM, EQ + 2 * EKV], BF, kind="ExternalInput")
    woT = nc.dram_tensor("woT", [DIM, EQ], BF, kind="ExternalInput")  # Wo[oc_slice,:].T
    mask128 = nc.dram_tensor("mask128", [128, 128], F32, kind="ExternalInput")
    ident = nc.dram_tensor("ident", [128, 128], BF, kind="ExternalInput")
    ones_in = nc.dram_tensor("ones_in", [128, 1], BF, kind="ExternalInput")
    out_part = nc.dram_tensor("out_part", [EQ, T], F32, kind="ExternalOutput")

    EALL = EQ + 2 * EKV  # 1536, 12 e-tiles: 8 Q, 2 K, 2 V
    NE = EALL // 128

    with tile.TileContext(nc) as tc:
        with (
            tc.tile_pool(name="persist", bufs=1) as persist,
            tc.tile_pool(name="stream", bufs=6) as stream,
            tc.tile_pool(name="work", bufs=3) as work,
            tc.tile_pool(name="dram2", bufs=1, space="DRAM") as dram2,
        ):
            # ---------------- constants ----------------
            mask_sb = persist.tile([128, 128], F32)
            nc.sync.dma_start(out=mask_sb[:], in_=mask128[:, :])
            ident_sb = persist.tile([128, 128], BF)
            nc.sync.dma_start(out=ident_sb[:], in_=ident[:, :])
            ones_sb = persist.tile([128, 1], BF)
            nc.sync.dma_start(out=ones_sb[:], in_=ones_in[:, :])

            # persistent activations
            qt_sb = persist.tile([128, HPC * T], BF)  # QT: head h at cols [h*T,(h+1)*T)
            kt_sb = persist.tile([128, KVPC * T], BF)  # KT per kv head
            vt_sb = persist.tile([128, KVPC * T], BF)  # VT per kv head
            v_sb = persist.tile([128, KVPC * T], BF)  # V[t,dv]: tile (g,kb) at (g*16+kb)*128

            # per-head AllGather buffers
            og_in = []
            og_out = []
            for h in range(HPC):
                oin = dram2.tile([128, T], BF, name=f"og_in_{h}")
                oout = dram2.tile([4 * 128, T], BF, name=f"og_out_{h}")
                og_in.append(oin)
                og_out.append(oout)

            with (
                tc.tile_pool(name="wall_pool", bufs=1) as wall_pool,
                tc.tile_pool(name="psum_p1", bufs=2, space="PSUM") as psum_p1,
            ):
                # phase-1 weights: c-tile cb at cols [cb*EALL, (cb+1)*EALL)
                wall_sb = wall_pool.tile([128, NC * EALL], BF)

                def load_wall(cb):
                    nc.sync.dma_start(
                        out=wall_sb[:, cb * EALL:(cb + 1) * EALL],
                        in_=wallT[cb * 128:(cb + 1) * 128, :],
                    )

                # ---------------- phase 1: projections ----------------
                # e-tile order: K0 K1 V0 V1 first so attention deps clear early
                etile_order = [HPC, HPC + 1, HPC + 2, HPC + 3] + list(range(HPC))

                def etile_dst(e):
                    # e indexes [Q0..Q7, K0, K1, V0, V1]
                    if e < HPC:
                        return qt_sb[:, e * T:(e + 1) * T]
                    if e < HPC + KVPC:
                        g = e - HPC
                        return kt_sb[:, g * T:(g + 1) * T]
                    g = e - HPC - KVPC
                    return vt_sb[:, g * T:(g + 1) * T]

                for chunk in range(3):  # 3 chunks of 4 e-tiles
                    es = etile_order[chunk * 4:(chunk + 1) * 4]
                    for tg in range(NT):
                        if chunk == 0 and tg == 0:
                            load_wall(0)
                        accs = []
                        for i, e in enumerate(es):
                            acc = psum_p1.tile([128, 512], F32, tag=f"acc{i}")
                            accs.append(acc)
                        for cb in range(NC):
                            if chunk == 0 and tg == 0 and cb + 1 < NC:
                                load_wall(cb + 1)
                            xt_t = stream.tile([128, 512], BF, tag="xt")
                            nc.sync.dma_start(
                                out=xt_t[:],
                                in_=xT[cb * 128:(cb + 1) * 128,
                                       tg * 512:(tg + 1) * 512],
                            )
                            for i, e in enumerate(es):
                                nc.tensor.matmul(
                                    accs[i][:],
                                    wall_sb[:, cb * EALL + e * 128:
                                            cb * EALL + (e + 1) * 128],
                                    xt_t[:],
                                    start=(cb == 0),
                                    stop=(cb == NC - 1),
                                )
                        for i, e in enumerate(es):
                            nc.vector.tensor_copy(
                                etile_dst(e)[:, tg * 512:(tg + 1) * 512], accs[i][:]
                            )

                # V = VT.T per 128x128 tile (PE transpose-mode; psum dtype = input)
                for g in range(KVPC):
                    for kb in range(NKB):
                        tp = psum_p1.tile([128, 128], BF, tag="acc0")
                        nc.tensor.transpose(
                            tp[:],
                            vt_sb[:, g * T + kb * 128:g * T + (kb + 1) * 128],
                            ident_sb[:],
                        )
                        nc.vector.tensor_copy(
                            v_sb[:, (g * NKB + kb) * 128:(g * NKB + kb + 1) * 128],
                            tp[:],
                        )

            # wall_pool/psum_p1 released; phase 2/3 reuse that SBUF/PSUM space.
            with (
                tc.tile_pool(name="p23", bufs=1) as p23,
                tc.tile_pool(name="work2", bufs=3) as work2,
            ):
                oT_sb = p23.tile([128, HPC * T], BF)  # local oT: head h at [h*T,..)
                woT_sb = p23.tile([128, NC * EQ], BF)  # phase-3 lhsT tiles
                for cb in range(NC):
                    nc.sync.dma_start(
                        out=woT_sb[:, cb * EQ:(cb + 1) * EQ],
                        in_=woT[cb * 128:(cb + 1) * 128, :],
                    )

                # ---------------- phase 2: attention ----------------
                with (
                    tc.tile_pool(name="ps_sT", bufs=3, space="PSUM") as ps_sT,
                    tc.tile_pool(name="ps_oT", bufs=2, space="PSUM") as ps_oT,
                    tc.tile_pool(name="ps_den", bufs=2, space="PSUM") as ps_den,
                ):
                    for h in range(HPC):
                        g = h // R  # local kv head
                        qt_h = qt_sb[:, h * T:(h + 1) * T]
                        kt_g = kt_sb[:, g * T:(g + 1) * T]
                        for tg in range(NT):
                            nkb = 4 * tg + 4  # causal: k-tiles 0..nkb-1
                            oT_acc = ps_oT.tile([128, 512], F32, tag="oT")
                            den_acc = ps_den.tile([1, 512], F32, tag="den")
                            for kb in range(nkb):
                                sT = ps_sT.tile([128, 512], F32, tag="sT")
                                nc.tensor.matmul(
                                    sT[:],
                                    kt_g[:, kb * 128:(kb + 1) * 128],
                                    qt_h[:, tg * 512:(tg + 1) * 512],
                                    start=True,
                                    stop=True,
                                )
                                jdiag = kb - 4 * tg  # diag 128-subtile (if 0..3)
                                jstart = max(0, jdiag)
                                if 0 <= jdiag < 4:
                                    nc.vector.tensor_tensor(
                                        sT[:, jdiag * 128:(jdiag + 1) * 128],
                                        sT[:, jdiag * 128:(jdiag + 1) * 128],
                                        mask_sb[:],
                                        mybir.AluOpType.add,
                                    )
                                expT = work2.tile([128, 512], BF, tag="expT", bufs=4)
                                if jstart > 0:
                                    nc.vector.memset(expT[:, :jstart * 128], 0.0)
                                nc.scalar.activation(
                                    expT[:, jstart * 128:],
                                    sT[:, jstart * 128:],
                                    mybir.ActivationFunctionType.Exp,
                                    scale=INV_SQRT_HD,
                                )
                                nc.tensor.matmul(
                                    den_acc[:],
                                    ones_sb[:],
                                    expT[:],
                                    start=(kb == 0),
                                    stop=(kb == nkb - 1),
                                    skip_group_check=True,
                                )
                                nc.tensor.matmul(
                                    oT_acc[:],
                                    v_sb[:, (g * NKB + kb) * 128:
                                         (g * NKB + kb + 1) * 128],
                                    expT[:],
                                    start=(kb == 0),
                                    stop=(kb == nkb - 1),
                                    skip_group_check=True,
                                )
                            recip = work2.tile([1, 512], F32, tag="recip")
                            nc.vector.reciprocal(recip[:], den_acc[:])
                            recip_b = work2.tile([128, 512], F32, tag="recip_b")
                            nc.gpsimd.partition_broadcast(recip_b[:], recip[:])
                            nc.vector.tensor_tensor(
                                oT_sb[:, h * T + tg * 512:h * T + (tg + 1) * 512],
                                oT_acc[:],
                                recip_b[:],
                                mybir.AluOpType.mult,
                            )
                        # ship this head's oT and gather peers'
                        nc.sync.dma_start(
                            out=og_in[h][:], in_=oT_sb[:, h * T:(h + 1) * T]
                        )
                        nc.gpsimd.collective_compute(
                            "AllGather",
                            mybir.AluOpType.bypass,
                            replica_groups=GROUPS,
                            ins=[og_in[h].opt()],
                            outs=[og_out[h].opt()],
                        )

                # ---------------- phase 3: outT slice = WoT.T @ oT_full --------
                # global e-tile eb <-> global head H: rank r = eb//8, local hl = eb%8
                with tc.tile_pool(name="ps_out", bufs=2, space="PSUM") as ps_out:
                    for tg in range(NT):
                        for occ in range(2):  # oc chunks of 4
                            accs = []
                            for oi in range(4):
                                acc = ps_out.tile([128, 512], F32, tag=f"out{oi}")
                                accs.append(acc)
                            eb_avail = [rr * HPC + hh
                                        for hh in range(HPC) for rr in range(4)]
                            for ei, eb in enumerate(eb_avail):  # 32 global e-tiles
                                r, hl = eb // HPC, eb % HPC
                                rhs_t = work2.tile([128, 512], BF, tag="rhs", bufs=6)
                                nc.sync.dma_start(
                                    out=rhs_t[:],
                                    in_=og_out[hl][r * 128:(r + 1) * 128,
                                                   tg * 512:(tg + 1) * 512],
                                )
                                for oi in range(4):
                                    oc = occ * 4 + oi
                                    nc.tensor.matmul(
                                        accs[oi][:],
                                        woT_sb[:, eb * EQ + oc * 128:
                                               eb * EQ + (oc + 1) * 128],
                                        rhs_t[:],
                                        start=(ei == 0),
                                        stop=(ei == NC - 1),
                                    )
                            for oi in range(4):
                                oc = occ * 4 + oi
                                ev = work2.tile([128, 512], F32, tag="ev")
                                nc.vector.tensor_copy(ev[:], accs[oi][:])
                                nc.sync.dma_start(
                                    out=out_part[oc * 128:(oc + 1) * 128,
                                                 tg * 512:(tg + 1) * 512],
                                    in_=ev[:],
                                )
    nc.finalize()
    return nc


_NC_CACHE = None


def _get_nc():
    global _NC_CACHE
    if _NC_CACHE is None:
        _NC_CACHE = build()
    return _NC_CACHE


def kernel(x, Wq, Wkv, Wo):
    x = np.asarray(x, dtype=np.float32)
    Wq = np.asarray(Wq, dtype=np.float32)
    Wkv = np.asarray(Wkv, dtype=np.float32)
    Wo = np.asarray(Wo, dtype=np.float32)

    # host-side prep (transposes + bf16 casts)
    try:
        import ml_dtypes

        bf16 = ml_dtypes.bfloat16
    except ImportError:  # pragma: no cover
        import jax.numpy as jnp

        bf16 = jnp.bfloat16

    xT_b = [np.ascontiguousarray(x[b].T).astype(bf16) for b in range(B)]

    mask = np.where(
        np.arange(128)[:, None] <= np.arange(128)[None, :], 0.0, -1e30
    ).astype(np.float32)  # [k,q]: allow k<=q
    ident = np.eye(128, dtype=np.float32).astype(bf16)
    ones = np.ones((128, 1), dtype=np.float32).astype(bf16)

    in_maps = []
    for c in range(NCORES):
        b, j = c // 4, c % 4
        wq_l = Wq[EQ * j:EQ * (j + 1), :]  # [1024, 4096]
        wk_l = Wkv[EKV * j:EKV * (j + 1), :]  # [256, 4096]
        wv_l = Wkv[N_KV * HD + EKV * j:N_KV * HD + EKV * (j + 1), :]
        wall = np.concatenate([wq_l, wk_l, wv_l], axis=0)  # [1536, 4096]
        wallT = np.ascontiguousarray(wall.T).astype(bf16)  # [4096, 1536]
        woT_l = np.ascontiguousarray(Wo[EQ * j:EQ * (j + 1), :].T).astype(bf16)
        in_maps.append(
            {
                "xT": xT_b[b],
                "wallT": wallT,
                "woT": woT_l,
                "mask128": mask,
                "ident": ident,
                "ones_in": ones,
            }
        )

    nc = _get_nc()
    res = run_bass_kernel_spmd(nc, in_maps, core_ids=list(range(NCORES)))

    out = np.empty((B, T, DIM), dtype=np.float32)
    for b in range(B):
        outT = np.concatenate(
            [res.results[b * 4 + j]["out_part"] for j in range(4)], axis=0
        )  # [4096, 2048]
        out[b] = outT.T
    return out



# revision 5
# speedup vs baseline: 1.1031x; 1.1031x over previous
"""GroupedQueryAttention on 8 Trainium2 NeuronCores — v2.

Problem (hardcoded): B=2, T=2048, DIM=4096, 32 q heads, 8 kv heads, hd=128.
  q = x @ Wq.T ; k,v = split(x @ Wkv.T) ; causal softmax(q k^T/sqrt(hd)) v ; out = o @ Wo.T

Sharding: hybrid data x tensor parallel over 8 cores.
  core c -> batch b = c//4, kv-head group j = c%4 (kv heads {2j,2j+1}, q heads {8j..8j+7}).

v2 changes over v1 (baseline 1.17 ms):
  P1: psum layout 4 e-tiles x 2 t-groups (8 banks, bufs=1); consecutive matmuls
      share the stationary weight (LDWEIGHTS amortization); xt prefetch depth 8.
  P2: score tiles paired into [128,1024] psum (2 banks); one Exp per pair;
      causal masking via multiplicative bf16 masks AFTER exp (2 fixed patterns);
      AV matmuls lag one pair behind (tensor never FIFO-stalls on exp);
      softmax denominator from an expsum accumulator (1 vector add per pair,
      2 tensor matmuls per (h,tg)) instead of 1 matmul per k-tile.
  P3: psum 4 oc x 2 tg (8 banks); og rhs strips [128,1024] loaded once per pass;
      weight reuse pairs; head-major accumulation order; head 7's AllGather
      split into two halves so the collective tail hides under P3's first pass.
"""

import sys

sys.path.insert(0, "/opt/trn_rl_repo")

import math

import numpy as np

import concourse.bass as bass
import concourse.bacc as bacc
import concourse.tile as tile
from concourse import mybir
from concourse.bass_utils import run_bass_kernel_spmd

B, T, DIM = 2, 2048, 4096
N_HEADS, N_KV, HD = 32, 8, 128
R = N_HEADS // N_KV  # 4
NCORES = 8
GROUPS = [[0, 1, 2, 3], [4, 5, 6, 7]]

HPC = 8  # q heads per core
KVPC = 2  # kv heads per core
EQ = HPC * HD  # 1024 q-proj out features per core
EKV = KVPC * HD  # 256 k (and v) out features per core
NT = T // 512  # 4 t-groups of 512
NC = DIM // 128  # 32 contraction tiles
NKB = T // 128  # 16 k-tiles per head

BF = mybir.dt.bfloat16
F32 = mybir.dt.float32
INV_SQRT_HD = 1.0 / math.sqrt(HD)


def build():
    nc = bacc.Bacc("TRN2", num_devices=NCORES)

    # ---- external I/O (per-core data differs, program is SPMD-identical) ----
    xT = nc.dram_tensor("xT", [DIM, T], BF, kind="ExternalInput")  # x[b].T
    wallT = nc.dram_tensor("wallT", [DIM, EQ + 2 * EKV], BF, kind="ExternalInput")
    woT = nc.dram_tensor("woT", [DIM, EQ], BF, kind="ExternalInput")  # Wo[oc_slice,:].T
    maskA = nc.dram_tensor("maskA", [128, 1024], BF, kind="ExternalInput")
    maskB = nc.dram_tensor("maskB", [128, 1024], BF, kind="ExternalInput")
    ident = nc.dram_tensor("ident", [128, 128], BF, kind="ExternalInput")
    ones_in = nc.dram_tensor("ones_in", [128, 1], BF, kind="ExternalInput")
    out_part = nc.dram_tensor("out_part", [EQ, T], F32, kind="ExternalOutput")

    EALL = EQ + 2 * EKV  # 1536, 12 e-tiles: 8 Q, 2 K, 2 V
    NE = EALL // 128

    with tile.TileContext(nc) as tc:
        with (
            tc.tile_pool(name="persist", bufs=1) as persist,
            tc.tile_pool(name="stream", bufs=8) as stream,
            tc.tile_pool(name="work", bufs=3) as work,
            tc.tile_pool(name="dram2", bufs=1, space="DRAM") as dram2,
        ):
            # ---------------- constants ----------------
            maskA_sb = persist.tile([128, 1024], BF)
            nc.sync.dma_start(out=maskA_sb[:], in_=maskA[:, :])
            maskB_sb = persist.tile([128, 1024], BF)
            nc.sync.dma_start(out=maskB_sb[:], in_=maskB[:, :])
            ident_sb = persist.tile([128, 128], BF)
            nc.sync.dma_start(out=ident_sb[:], in_=ident[:, :])
            ones_sb = persist.tile([128, 1], BF)
            nc.sync.dma_start(out=ones_sb[:], in_=ones_in[:, :])

            # persistent activations
            qt_sb = persist.tile([128, HPC * T], BF)  # QT: head h at cols [h*T,(h+1)*T)
            kt_sb = persist.tile([128, KVPC * T], BF)  # KT per kv head
            vt_sb = persist.tile([128, KVPC * T], BF)  # VT per kv head
            v_sb = persist.tile([128, KVPC * T], BF)  # V[t,dv]: tile (g,kb) at (g*16+kb)*128

            # per-head AllGather buffers (head 7 split in halves)
            og_in = []
            og_out = []
            for h in range(HPC - 1):
                og_in.append(dram2.tile([128, T], BF, name=f"og_in_{h}"))
                og_out.append(dram2.tile([4 * 128, T], BF, name=f"og_out_{h}"))
            og_in7 = [dram2.tile([128, 1024], BF, name=f"og_in7_{i}",
                                 tag=f"og_in7_{i}") for i in range(2)]
            og_out7 = [dram2.tile([4 * 128, 1024], BF, name=f"og_out7_{i}",
                                  tag=f"og_out7_{i}") for i in range(2)]

            with (
                tc.tile_pool(name="wall_pool", bufs=1) as wall_pool,
                tc.tile_pool(name="psum_p1", bufs=1, space="PSUM") as psum_p1,
            ):
                # phase-1 weights: c-tile cb at cols [cb*EALL, (cb+1)*EALL)
                wall_sb = wall_pool.tile([128, NC * EALL], BF)

                def load_wall(cb):
                    nc.sync.dma_start(
                        out=wall_sb[:, cb * EALL:(cb + 1) * EALL],
                        in_=wallT[cb * 128:(cb + 1) * 128, :],
                    )

                # ---------------- phase 1: projections ----------------
                # e-tile order: K0 K1 V0 V1 first so attention deps clear early
                def etile_dst(e):
                    # e indexes [Q0..Q7, K0, K1, V0, V1]
                    if e < HPC:
                        return qt_sb[:, e * T:(e + 1) * T]
                    if e < HPC + KVPC:
                        g = e - HPC
                        return kt_sb[:, g * T:(g + 1) * T]
                    g = e - HPC - KVPC
                    return vt_sb[:, g * T:(g + 1) * T]

                chunks = [[HPC, HPC + 1, HPC + 2, HPC + 3],
                          [0, 1, 2, 3], [4, 5, 6, 7]]
                for ci, es in enumerate(chunks):
                    for tgp in range(2):  # t-group pairs {0,1}, {2,3}
                        accs = [[psum_p1.tile([128, 512], F32, tag=f"acc{i}{j}",
                                              name=f"acc{i}{j}")
                                 for j in range(2)] for i in range(4)]
                        for cb in range(NC):
                            if ci == 0 and tgp == 0 and cb == 0:
                                load_wall(0)
                            if ci == 0 and tgp == 0 and cb + 1 < NC:
                                load_wall(cb + 1)
                            xts = []
                            for j in range(2):
                                t0 = (2 * tgp + j) * 512
                                xt_t = stream.tile([128, 512], BF, tag="xt")
                                nc.sync.dma_start(
                                    out=xt_t[:],
                                    in_=xT[cb * 128:(cb + 1) * 128, t0:t0 + 512],
                                )
                                xts.append(xt_t)
                            for i, e in enumerate(es):
                                w = wall_sb[:, cb * EALL + e * 128:
                                            cb * EALL + (e + 1) * 128]
                                for j in range(2):
                                    nc.tensor.matmul(
                                        accs[i][j][:], w, xts[j][:],
                                        start=(cb == 0), stop=(cb == NC - 1),
                                    )
                        for i, e in enumerate(es):
                            for j in range(2):
                                t0 = (2 * tgp + j) * 512
                                nc.vector.tensor_copy(
                                    etile_dst(e)[:, t0:t0 + 512], accs[i][j][:]
                                )
                    if ci == 0:
                        # V = VT.T per 128x128 tile (PE transpose-mode)
                        for g in range(KVPC):
                            for kb in range(NKB):
                                tp = psum_p1.tile([128, 128], BF,
                                                  tag=f"acc{kb % 4}{g}")
                                nc.tensor.transpose(
                                    tp[:],
                                    vt_sb[:, g * T + kb * 128:
                                          g * T + (kb + 1) * 128],
                                    ident_sb[:],
                                )
                                nc.vector.tensor_copy(
                                    v_sb[:, (g * NKB + kb) * 128:
                                         (g * NKB + kb + 1) * 128],
                                    tp[:],
                                )

            # wall_pool/psum_p1 released; phase 2/3 reuse that SBUF/PSUM space.
            with (
                tc.tile_pool(name="p23", bufs=1) as p23,
                tc.tile_pool(name="work2", bufs=3) as work2,
            ):
                oT_sb = p23.tile([128, HPC * T], BF)  # local oT: head h at [h*T,..)
                woT_sb = p23.tile([128, NC * EQ], BF)  # phase-3 lhsT tiles
                for cb in range(NC):
                    nc.sync.dma_start(
                        out=woT_sb[:, cb * EQ:(cb + 1) * EQ],
                        in_=woT[cb * 128:(cb + 1) * 128, :],
                    )

                # ---------------- phase 2: attention ----------------
                with (
                    tc.tile_pool(name="ps_sT", bufs=2, space="PSUM") as ps_sT,
                    tc.tile_pool(name="ps_oT", bufs=2, space="PSUM") as ps_oT,
                    tc.tile_pool(name="ps_den", bufs=2, space="PSUM") as ps_den,
                ):
                    for h in range(HPC):
                        g = h // R  # local kv head
                        qt_h = qt_sb[:, h * T:(h + 1) * T]
                        kt_g = kt_sb[:, g * T:(g + 1) * T]
                        for tg in range(NT):
                            npairs = 2 * tg + 2  # k-tile pairs 0..npairs-1
                            qs = qt_h[:, tg * 512:(tg + 1) * 512]
                            oT_acc = ps_oT.tile([128, 512], F32, tag="oT")
                            expsum = work2.tile([128, 1024], BF, tag="expsum",
                                                bufs=2)
                            expp_prev = None

                            def emit_av(p, expp):
                                first = (p == 0)
                                last = (p == npairs - 1)
                                for j in range(2):
                                    kb = 2 * p + j
                                    nc.tensor.matmul(
                                        oT_acc[:],
                                        v_sb[:, (g * NKB + kb) * 128:
                                             (g * NKB + kb + 1) * 128],
                                        expp[:, j * 512:(j + 1) * 512],
                                        start=(first and j == 0),
                                        stop=(last and j == 1),
                                        skip_group_check=True,
                                    )

                            for p in range(npairs):
                                sT2 = ps_sT.tile([128, 1024], F32, tag="sT2")
                                for j in range(2):
                                    kb = 2 * p + j
                                    nc.tensor.matmul(
                                        sT2[:, j * 512:(j + 1) * 512],
                                        kt_g[:, kb * 128:(kb + 1) * 128],
                                        qs,
                                        start=True, stop=True,
                                        skip_group_check=True,
                                    )
                                expp = work2.tile([128, 1024], BF, tag="expT2",
                                                  bufs=3)
                                nc.scalar.activation(
                                    expp[:], sT2[:],
                                    mybir.ActivationFunctionType.Exp,
                                    scale=INV_SQRT_HD,
                                )
                                # diagonal pairs: multiplicative causal mask
                                if p == npairs - 2:
                                    nc.vector.tensor_tensor(
                                        expp[:], expp[:], maskA_sb[:],
                                        mybir.AluOpType.mult,
                                    )
                                elif p == npairs - 1:
                                    nc.vector.tensor_tensor(
                                        expp[:], expp[:], maskB_sb[:],
                                        mybir.AluOpType.mult,
                                    )
                                # expsum accumulate (bf16, [128,1024])
                                if p == 0:
                                    nc.vector.tensor_copy(expsum[:], expp[:])
                                else:
                                    nc.vector.tensor_tensor(
                                        expsum[:], expsum[:], expp[:],
                                        mybir.AluOpType.add,
                                    )
                                # AV for previous pair (tensor stays 1 pair
                                # ahead of the exp dependency)
                                if expp_prev is not None:
                                    emit_av(p - 1, expp_prev)
                                expp_prev = expp
                            emit_av(npairs - 1, expp_prev)

                            # denominator: [1,512] = ones.T @ (sumL + sumR)
                            den_acc = ps_den.tile([1, 512], F32, tag="den")
                            for j in range(2):
                                nc.tensor.matmul(
                                    den_acc[:], ones_sb[:],
                                    expsum[:, j * 512:(j + 1) * 512],
                                    start=(j == 0), stop=(j == 1),
                                    skip_group_check=True,
                                )
                            recip = work2.tile([1, 512], F32, tag="recip")
                            nc.vector.reciprocal(recip[:], den_acc[:])
                            recip_b = work2.tile([128, 512], F32, tag="recip_b")
                            nc.gpsimd.partition_broadcast(recip_b[:], recip[:])
                            nc.vector.tensor_tensor(
                                oT_sb[:, h * T + tg * 512:h * T + (tg + 1) * 512],
                                oT_acc[:],
                                recip_b[:],
                                mybir.AluOpType.mult,
                            )
                            # head 7 ships in halves so its AllGather tail
                            # overlaps phase 3's first pass
                            if h == HPC - 1 and tg in (1, 3):
                                i7 = tg // 2
                                nc.sync.dma_start(
                                    out=og_in7[i7][:],
                                    in_=oT_sb[:, h * T + i7 * 1024:
                                              h * T + (i7 + 1) * 1024],
                                )
                                nc.gpsimd.collective_compute(
                                    "AllGather",
                                    mybir.AluOpType.bypass,
                                    replica_groups=GROUPS,
                                    ins=[og_in7[i7].opt()],
                                    outs=[og_out7[i7].opt()],
                                )
                        if h < HPC - 1:
                            nc.sync.dma_start(
                                out=og_in[h][:], in_=oT_sb[:, h * T:(h + 1) * T]
                            )
                            nc.gpsimd.collective_compute(
                                "AllGather",
                                mybir.AluOpType.bypass,
                                replica_groups=GROUPS,
                                ins=[og_in[h].opt()],
                                outs=[og_out[h].opt()],
                            )

                # ---------------- phase 3: outT slice = WoT.T @ oT_full --------
                # accumulate e-tiles head-major (AG completion order), head 7
                # last; psum = 4 oc x 2 tg halves (8 banks), og strips loaded
                # once per pass.
                eb_order = [rr * HPC + hh for hh in range(HPC) for rr in range(4)]
                with tc.tile_pool(name="ps_out", bufs=1, space="PSUM") as ps_out:
                    for ocp in range(2):
                        for tgp in range(2):
                            accs = [[ps_out.tile([128, 512], F32,
                                                 tag=f"out{oi}{j}",
                                                 name=f"out{oi}{j}")
                                     for j in range(2)] for oi in range(4)]
                            for ei, eb in enumerate(eb_order):
                                r, hl = eb // HPC, eb % HPC
                                rhs_t = work2.tile([128, 1024], BF, tag="rhs",
                                                   bufs=6)
                                if hl == HPC - 1:
                                    src = og_out7[tgp][r * 128:(r + 1) * 128, :]
                                else:
                                    src = og_out[hl][r * 128:(r + 1) * 128,
                                                     tgp * 1024:(tgp + 1) * 1024]
                                nc.sync.dma_start(out=rhs_t[:], in_=src)
                                for oi in range(4):
                                    oc = ocp * 4 + oi
                                    w = woT_sb[:, eb * EQ + oc * 128:
                                               eb * EQ + (oc + 1) * 128]
                                    for j in range(2):
                                        nc.tensor.matmul(
                                            accs[oi][j][:],
                                            w,
                                            rhs_t[:, j * 512:(j + 1) * 512],
                                            start=(ei == 0),
                                            stop=(ei == NC - 1),
                                        )
                            for oi in range(4):
                                oc = ocp * 4 + oi
                                for j in range(2):
                                    t0 = tgp * 1024 + j * 512
                                    ev = work2.tile([128, 512], F32, tag="ev",
                                                    bufs=4)
                                    nc.vector.tensor_copy(ev[:], accs[oi][j][:])
                                    nc.sync.dma_start(
                                        out=out_part[oc * 128:(oc + 1) * 128,
                                                     t0:t0 + 512],
                                        in_=ev[:],
                                    )
    nc.finalize()
    return nc


_NC_CACHE = None


def _get_nc():
    global _NC_CACHE
    if _NC_CACHE is None:
        _NC_CACHE = build()
    return _NC_CACHE


def kernel(x, Wq, Wkv, Wo):
    x = np.asarray(x, dtype=np.float32)
    Wq = np.asarray(Wq, dtype=np.float32)
    Wkv = np.asarray(Wkv, dtype=np.float32)
    Wo = np.asarray(Wo, dtype=np.float32)

    # host-side prep (transposes + bf16 casts)
    try:
        import ml_dtypes

        bf16 = ml_dtypes.bfloat16
    except ImportError:  # pragma: no cover
        import jax.numpy as jnp

        bf16 = jnp.bfloat16

    xT_b = [np.ascontiguousarray(x[b].T).astype(bf16) for b in range(B)]

    # multiplicative causal masks for the two diagonal pair positions:
    # pair tile j covers k-tile jdiag = 2*pos + j; element (kl, j*512+ql)
    # is kept iff kl <= ql - 128*jdiag.
    kl = np.arange(128)[:, None]
    ql = np.arange(512)[None, :]
    masks = []
    for pos in range(2):
        cols = []
        for j in range(2):
            jd = 2 * pos + j
            cols.append((kl <= ql - 128 * jd).astype(np.float32))
        masks.append(np.concatenate(cols, axis=1).astype(bf16))
    maskA_np, maskB_np = masks

    ident = np.eye(128, dtype=np.float32).astype(bf16)
    ones = np.ones((128, 1), dtype=np.float32).astype(bf16)

    in_maps = []
    for c in range(NCORES):
        b, j = c // 4, c % 4
        wq_l = Wq[EQ * j:EQ * (j + 1), :]  # [1024, 4096]
        wk_l = Wkv[EKV * j:EKV * (j + 1), :]  # [256, 4096]
        wv_l = Wkv[N_KV * HD + EKV * j:N_KV * HD + EKV * (j + 1), :]
        wall = np.concatenate([wq_l, wk_l, wv_l], axis=0)  # [1536, 4096]
        wallT = np.ascontiguousarray(wall.T).astype(bf16)  # [4096, 1536]
        woT_l = np.ascontiguousarray(Wo[EQ * j:EQ * (j + 1), :].T).astype(bf16)
        in_maps.append(
            {
                "xT": xT_b[b],
                "wallT": wallT,
                "woT": woT_l,
                "maskA": maskA_np,
                "maskB": maskB_np,
                "ident": ident,
                "ones_in": ones,
            }
        )

    nc = _get_nc()
    res = run_bass_kernel_spmd(nc, in_maps, core_ids=list(range(NCORES)))

    out = np.empty((B, T, DIM), dtype=np.float32)
    for b in range(B):
        outT = np.concatenate(
            [res.results[b * 4 + j]["out_part"] for j in range(4)], axis=0
        )  # [4096, 2048]
        out[b] = outT.T
    return out
